# revision 36
# baseline (speedup 1.0000x reference)
"""Trainium2 Bass kernel for DecoderSplattingCUDA (EWA Gaussian splatting).

Contract: kernel(**inputs) takes the FULL inputs of reference.setup_inputs()
and returns the FULL [b, v, 3, H, W] image, computed on 8 NeuronCores.

Layout: gaussians on partitions (depth sorted), pixels on the free axis.
The image is split into 128 (camera, 8-row band, x-quarter) rects of
8x32 = 256 px, striped across the 8 cores (16 slots per core) by survivor
count for load balance.  Per rect the host culls gaussians that can reach
alpha >= 1/255 inside the rect (conservative in both y and x, so results
stay exact) and pads survivors to BPAD blocks of 128.  Blocks are
processed in chunks of 2 ([128 g, 512 px] elementwise ops) to amortize
per-instruction access bubbles.

Per chunk:
  D    = coef^T @ basis        (one fp16 matmul per block into a PSUM
                                half; coefficients are host-precomputed
                                3-way fp16 splits of the quadratic
                                A*x^2 + B_r*x + C_r so every product is
                                exact in the f32 accumulate -- fp32r
                                would round inputs to ~11 bits, which the
                                completed-square cancellation amplifies)
  a0   = exp(-D)               (scalar act, straight from PSUM)
  c1   = min(a0, 0.99)         (gpsimd; == the max(D, -ln.99) clamp)
  am   = (a0 >= 1/255) * c1    (one fused vector scalar_tensor_tensor)
  lga  = ln(1 - am)            (scalar act, f32r out)
Depth-ordered transmittance T_g = exp(cumsum lga) is a triangular-ones
fp32r matmul per block (1 cycle/row); carries across blocks come from a
staircase matmul accumulated over the rect's blocks and broadcast back
with a selector-row matmul.  f32r is safe here: its input rounding is
relative to log-T, so T keeps ~2^-11 relative accuracy.  The composite
uses summation by parts: img = c_0 + sum_g (c_{g+1}-c_g) T_g with
c_G := background, so the color matmul contracts T directly.

Emission interleaves phase C of slot sl-1 between the phase-A chunks of
slot sl, so each in-order engine sequencer has ready work from the other
stream while a chunk's serial PE->ACT->Pool->DVE->ACT chain progresses.
"""
import os
import sys

sys.path.insert(0, "/opt/trn_rl_repo/concourse")

from contextlib import ExitStack

import numpy as np

import concourse.bacc as bacc
import concourse.tile as tile
from concourse import mybir
from concourse.bass_utils import run_bass_kernel_spmd
from concourse.hw_specs import get_activation_tables

F32 = mybir.dt.float32
F32R = mybir.dt.float32r
F16 = mybir.dt.float16
AF = mybir.ActivationFunctionType
ALU = mybir.AluOpType

C0 = 0.28209479177387814
C1 = 0.4886025119029199
NEAR, FAR = 0.1, 1000.0

H = W = 128
G = 2048               # gaussians per camera (2 * 32 * 32)
NCAM = 2
BAND_ROWS = 8          # image rows per band
XW = 32                # columns per x-slice
NBAND = H // BAND_ROWS          # bands per camera (16)
NXS = W // XW                   # x-halves (2)
NPAIR = NCAM * NBAND * NXS      # rects (64)
NSLOT = NPAIR // 8              # rects per core (8)
BPX = BAND_ROWS * XW            # pixels per rect (512)
# D-matmul contraction rows: fp16 3-way-split coefficients so products are
# exact in the f32 PSUM accumulate (fp32r would round inputs to ~11 bits,
# which the completed-square cancellation amplifies).
# rows 0-4: A splits x {x2h, x2l} (A2*x2l dropped, ~2^-33 relative)
# rows 5+3r+s: B_r split s against basis x*1_r
# rows 29+3r+s: C_r split s against basis 1_r
NBAS = 5 + 3 * BAND_ROWS + 3 * BAND_ROWS   # 53

LN99 = float(np.float32(-np.log(np.float32(0.99))))     # 0.01005034
LN255 = float(np.float32(np.log(np.float32(255.0))))    # 5.5412636
INV255 = float(np.float32(1.0) / np.float32(255.0))     # 0.00392157
NEG_BIG = -200.0
PAD_C = 1000.0          # padding rows: D = 1000 -> alpha = 0

_NC_CACHE = {}
_LAST_EXEC_NS = None
_LAST_RESULTS = None


def _only_full_act_set(arch):
    """Steer insert_act_table_loads to the one table set that covers
    Exp+Ln+Copy+Identity (natural_log_exp_and_others), so the kernel pays a
    single ACT table load instead of one per function switch."""
    full = get_activation_tables(arch)
    keep = "natural_log_exp_and_others"
    return {name: (fns if name == keep else set()) for name, fns in full.items()}


# ---------------------------------------------------------------- host prep
def _prep_camera(extr, K, bg, means, cov, sh, op):
    """Mirror of reference._render_one's per-gaussian math (numpy f32).
    Returns depth-sorted per-gaussian arrays."""
    f32 = np.float32
    extr = extr.astype(f32)
    try:
        w2c = np.linalg.inv(extr.astype(np.float64)).astype(f32)
    except np.linalg.LinAlgError:
        w2c = np.linalg.pinv(extr.astype(np.float64)).astype(f32)
    R, t = w2c[:3, :3], w2c[:3, 3]
    p = means @ R.T + t
    x, y, z = p[:, 0], p[:, 1], p[:, 2]
    zc = np.maximum(z, f32(1e-6))
    fx, fy = K[0, 0], K[1, 1]
    cx, cy = K[0, 2], K[1, 2]
    u = fx * x / zc + cx
    v = fy * y / zc + cy
    cov_c = np.einsum("ij,gjk,lk->gil", R, cov, R)
    zero = np.zeros_like(zc)
    J = np.stack([np.stack([fx / zc, zero, -fx * x / (zc * zc)], -1),
                  np.stack([zero, fy / zc, -fy * y / (zc * zc)], -1)], -2)
    cov2d = np.einsum("gij,gjk,glk->gil", J, cov_c, J)
    a = cov2d[:, 0, 0] + f32(0.3)
    bb = cov2d[:, 0, 1]
    c = cov2d[:, 1, 1] + f32(0.3)
    det = np.maximum(a * c - bb * bb, f32(1e-12))
    ia, ib, ic = c / det, -bb / det, a / det
    # SH degree-1 -> RGB
    d = means - extr[:3, 3]
    d = d / np.linalg.norm(d, axis=-1, keepdims=True)
    col = C0 * sh[:, :, 0]
    if sh.shape[-1] >= 4:
        col = (col - C1 * d[:, 1:2] * sh[:, :, 1]
               + C1 * d[:, 2:3] * sh[:, :, 2]
               - C1 * d[:, 0:1] * sh[:, :, 3])
    col = np.maximum(col + f32(0.5), f32(0.0)).astype(f32)  # [G, 3]

    valid = (z > f32(NEAR)) & (z < f32(FAR))
    op_eff = np.where(valid, op, f32(0.0))

    order = np.argsort(z, kind="stable")
    u, v, ia, ib, ic, op_eff, z = (arr[order] for arr in
                                   (u, v, ia, ib, ic, op_eff, z))
    col = col[order]

    # completed square: power = -sa*(gamma*(dx + r*dy))^2 - se*(delta*dy)^2
    psd = bool(np.all(ia > 0))
    with np.errstate(divide="ignore", invalid="ignore"):
        r = np.where(ia != 0, ib / ia, f32(0.0)).astype(f32)
        eta = ic - np.where(ia != 0, ib * ib / ia, f32(0.0))
        gamma = np.sqrt(np.abs(ia) * f32(0.5)).astype(f32)
        delta = np.sqrt(np.abs(eta) * f32(0.5)).astype(f32)
        logop = np.where(op_eff > 0, np.log(np.maximum(op_eff, f32(1e-30))),
                         f32(NEG_BIG))
    logop = np.maximum(logop, f32(NEG_BIG)).astype(f32)
    sa = np.sign(ia).astype(f32)
    sa[sa == 0] = 1.0
    se = np.sign(eta).astype(f32)
    se[se == 0] = 1.0
    psd = psd and bool(np.all(eta > 0))
    return dict(u=u.astype(f32), v=v.astype(f32), r=r, gamma=gamma,
                delta=delta, logop=logop, sa=sa, se=se, col=col,
                psd=psd, psd_g=(ia > 0) & (eta > 0))


def _cull_rect(cp, band, xh, bg):
    """Indices (in sorted order) of gaussians that can reach alpha >= 1/255
    anywhere in the rect; conservative in y and x, so dropped ones are
    exactly zero in the reference too.  Returns (idx, dc[3/kept], c0[3])."""
    f32 = np.float32
    ylo = f32(band * BAND_ROWS + 0.5)
    yhi = f32(band * BAND_ROWS + BAND_ROWS - 0.5)
    xlo = f32(xh * XW + 0.5)
    xhi = f32(xh * XW + XW - 0.5)
    v, u = cp["v"], cp["u"]
    dymin = np.maximum(0.0, np.maximum(ylo - v, v - yhi)).astype(f32)
    budget = cp["logop"] + f32(LN255 + 0.01)
    yterm = (cp["delta"] * dymin) ** 2
    keep = yterm <= budget
    # x-reach: s = x + r*dy - u is zero at x = u - r*dy; over the band's dy
    # range the zero sweeps an interval; distance from the rect to it bounds
    # |s| from below (conservative: continuous dy range contains row centers)
    sh = np.stack([cp["r"] * (ylo - v), cp["r"] * (yhi - v)])
    c_lo = u - sh.max(0)
    c_hi = u - sh.min(0)
    dxmin = np.maximum(0.0, np.maximum(xlo - c_hi, c_lo - xhi)).astype(f32)
    keep &= ((cp["gamma"] * dxmin) ** 2 + yterm) <= budget
    keep |= ~cp["psd_g"]     # non-PSD conics: never cull
    idx = np.nonzero(keep)[0]
    col = cp["col"][idx]
    n = len(idx)
    dc = np.zeros((n, 3), f32)
    if n:
        dc[:-1] = col[1:] - col[:-1]
        dc[-1] = bg - col[-1]
        c0 = col[0].copy()
    else:
        c0 = bg.astype(f32).copy()
    return idx, dc, c0


def _split3(v):
    """f32 -> three fp16 parts summing to ~33-bit precision."""
    f32, f16 = np.float32, np.float16
    v0 = v.astype(f16)
    r1 = (v - v0.astype(f32)).astype(f32)
    v1 = r1.astype(f16)
    v2 = (r1 - v1.astype(f32)).astype(f16)
    return v0, v1, v2


def _coef_block(cp, idx, band, xh):
    """Host-side D-matmul coefficients [NBAS, n] fp16 (3-way split) for one
    rect's survivors: D = A*x^2 + B_r*x + C_r per band row r, x local+0.5."""
    f32 = np.float32
    u_ = cp["u"][idx]
    r_ = cp["r"][idx]
    g_ = cp["gamma"][idx]
    v_ = cp["v"][idx]
    d_ = cp["delta"][idx]
    lo_ = cp["logop"][idx]
    sa = cp["sa"][idx]
    se = cp["se"][idx]
    dy = (np.arange(BAND_ROWS, dtype=f32) + band * BAND_ROWS
          + 0.5)[None, :] - v_[:, None]                     # [n, 8]
    A = (sa * g_ * g_).astype(f32)
    E = (r_[:, None] * dy - u_[:, None] + f32(xh * XW)).astype(f32)
    B = np.clip((2.0 * A[:, None] * E), -6e4, 6e4).astype(f32)
    C = np.clip((A[:, None] * E * E + (se * d_ * d_)[:, None] * dy * dy
                 - lo_[:, None]), -6e4, 6e4).astype(f32)
    n = len(idx)
    A0, A1, A2 = _split3(np.clip(A, -6e4, 6e4))
    B0, B1, B2 = _split3(B)    # [n, 8] each
    Cs = _split3(C)
    coef = np.zeros((NBAS, n), np.float16)
    coef[0] = A0
    coef[1] = A0
    coef[2] = A1
    coef[3] = A1
    coef[4] = A2
    for rr in range(BAND_ROWS):
        for s, Bs in enumerate((B0, B1, B2)):
            coef[5 + 3 * rr + s] = Bs[:, rr]
        for s in range(3):
            coef[5 + 3 * BAND_ROWS + 3 * rr + s] = Cs[s][:, rr]
    return coef


# ------------------------------------------------------------- bass program
def _build_nc(general: bool, bpads: tuple):
    nc = bacc.Bacc(None, target_bir_lowering=False)

    NBLK = sum(bpads)
    mb = max(bpads)
    koff = [sum(bpads[:i]) for i in range(NSLOT)]
    coef_d = nc.dram_tensor("coef", [NBAS, NBLK * 128], F16,
                            kind="ExternalInput")
    basis_d = nc.dram_tensor("basis", [NBAS, BPX], F16, kind="ExternalInput")
    dc_d = nc.dram_tensor("dcw", [128, NBLK * 3], F32R, kind="ExternalInput")
    u128_d = nc.dram_tensor("u128", [128, 128], F32R, kind="ExternalInput")
    st_d = nc.dram_tensor("st", [128, mb * mb], F32R, kind="ExternalInput")
    eb_d = nc.dram_tensor("eb", [mb, mb * 128], F32R, kind="ExternalInput")
    gs_d = nc.dram_tensor("gs", [128, NBLK], F32, kind="ExternalInput")
    img_d = nc.dram_tensor("img", [3, NSLOT * BPX], F32, kind="ExternalOutput")

    CPB = 512 // BPX        # blocks per elementwise chunk (PSUM: 1 bank)
    CW = CPB * BPX          # chunk width (512)

    with tile.TileContext(nc) as tc, ExitStack() as ctx:
        consts = ctx.enter_context(tc.tile_pool(name="consts", bufs=1))
        work = ctx.enter_context(tc.tile_pool(name="work", bufs=4))
        lgap = ctx.enter_context(
            tc.tile_pool(name="lgap", bufs=3 * (-(-mb // CPB))))
        carp = ctx.enter_context(tc.tile_pool(name="carp", bufs=2))
        outp = ctx.enter_context(tc.tile_pool(name="outp", bufs=2))
        dpsum = ctx.enter_context(tc.tile_pool(name="dpsum", bufs=1,
                                               space="PSUM"))
        cpsum = ctx.enter_context(tc.tile_pool(name="cpsum", bufs=1,
                                               space="PSUM"))
        spsum = ctx.enter_context(tc.tile_pool(name="spsum", bufs=1,
                                               space="PSUM"))
        ipsum = ctx.enter_context(tc.tile_pool(name="ipsum", bufs=1,
                                               space="PSUM"))

        # coef is large; split its load per-slot so the first chunks only
        # wait on their own slice, and spread loads over separate queues.
        coefs_t = [consts.tile([NBAS, bpads[s] * 128], F16, name=f"coef{s}")
                   for s in range(NSLOT)]
        basis = consts.tile([NBAS, BPX], F16)
        dcw = consts.tile([128, NBLK * 3], F32R)
        u128 = consts.tile([128, 128], F32R)
        st = consts.tile([128, mb * mb], F32R)
        eb = consts.tile([mb, mb * 128], F32R)
        gs = consts.tile([128, NBLK], F32)
        nc.sync.dma_start(basis[:], basis_d[:])
        nc.gpsimd.dma_start(u128[:], u128_d[:])
        nc.gpsimd.dma_start(dcw[:], dc_d[:])
        nc.gpsimd.dma_start(st[:], st_d[:])
        nc.gpsimd.dma_start(eb[:], eb_d[:])
        if general:
            nc.gpsimd.dma_start(gs[:], gs_d[:])
        for s in range(NSLOT):
            nc.sync.dma_start(
                coefs_t[s][:],
                coef_d[:, koff[s] * 128:(koff[s] + bpads[s]) * 128])

        def coef_ap(k):
            sl = max(s for s in range(NSLOT) if koff[s] <= k)
            b = k - koff[sl]
            return coefs_t[sl][:, b * 128:(b + 1) * 128]

        slot_ps_c = {}
        slot_lgas = {}

        def emit_A_front(sl, c0):
            """D matmuls + Exp for one chunk; returns pend for the back."""
            bpad = bpads[sl]
            if sl not in slot_ps_c:
                slot_ps_c[sl] = (cpsum.tile([128, BPX], F32,
                                            tag=f"ps_c{sl % 2}",
                                            name=f"ps_c{sl}")
                                 if bpad > 1 else None)
                slot_lgas[sl] = []
            blocks = list(range(c0, min(c0 + CPB, bpad)))
            cwid = len(blocks) * BPX
            ps_d = dpsum.tile([128, CW], F32, tag=f"ps_d{(c0 // CPB) % 2}",
                              name=f"ps_d{sl}_{c0}")
            for j, b in enumerate(blocks):
                k = koff[sl] + b
                nc.tensor.matmul(ps_d[:, j * BPX:(j + 1) * BPX],
                                 coef_ap(k),
                                 basis[:], start=True, stop=True)
            # a0 = exp(-D) straight from PSUM; the 0.99 clamp moves to a
            # Pool min (max(D, ln99) pre-exp == min(a0, 0.99) post-exp)
            # and the 1/255 cull is one fused DVE stt on a0 itself.
            a0 = work.tile([128, CW], F32, tag="a0")
            if general:
                # guard: non-PSD D can be hugely negative -> exp inf;
                # clamp D first (slower, correctness-only path)
                Dc = work.tile([128, CW], F32, tag="Dc")
                nc.vector.tensor_scalar(Dc[:, :cwid], ps_d[:, :cwid],
                                        LN99, None, ALU.max)
                nc.scalar.activation(a0[:, :cwid], Dc[:, :cwid],
                                     AF.Exp, scale=-1.0)
            else:
                nc.scalar.activation(a0[:, :cwid], ps_d[:, :cwid],
                                     AF.Exp, scale=-1.0)
            return (sl, blocks, cwid, ps_d, a0)

        def emit_A_back(pend):
            """min/stt/Ln/stair for a front-stage chunk; finishes the slot's
            phase B when it was the slot's last chunk."""
            sl, blocks, cwid, ps_d, a0 = pend
            bpad = bpads[sl]
            ps_c = slot_ps_c[sl]
            lgas = slot_lgas[sl]
            c1 = work.tile([128, CW], F32, tag="c1")
            nc.gpsimd.tensor_scalar(c1[:, :cwid], a0[:, :cwid], 0.99,
                                    None, ALU.min)
            am = work.tile([128, CW], F32, tag="am")
            nc.vector.scalar_tensor_tensor(am[:, :cwid], a0[:, :cwid],
                                           INV255, c1[:, :cwid],
                                           ALU.is_ge, ALU.mult)
            if general:
                am2 = work.tile([128, CW], F32, tag="am2")
                for j, b in enumerate(blocks):
                    k = koff[sl] + b
                    nc.vector.scalar_tensor_tensor(
                        am2[:, j * BPX:(j + 1) * BPX],
                        ps_d[:, j * BPX:(j + 1) * BPX],
                        gs[:, k:k + 1], am[:, j * BPX:(j + 1) * BPX],
                        ALU.is_ge, ALU.mult)
                am = am2
            lga = lgap.tile([128, CW], F32R, tag="lga")
            nc.scalar.activation(lga[:, :cwid], am[:, :cwid], AF.Ln,
                                 scale=-1.0, bias=1.0)
            for j, b in enumerate(blocks):
                lgas.append(lga[:, j * BPX:(j + 1) * BPX])
                if bpad > 1 and b < bpad - 1:
                    nc.tensor.matmul(ps_c[:bpad, :],
                                     st[:, mb * b:mb * b + bpad],
                                     lga[:, j * BPX:(j + 1) * BPX],
                                     start=(b == 0),
                                     stop=(b == bpad - 2))
            if blocks[-1] == bpad - 1:
                # phase B: carries to SBUF (f32, no compensation needed)
                if bpad > 1:
                    cw = carp.tile([128, BPX], F32R, tag="cw")
                    nc.vector.tensor_copy(cw[:bpad, :], ps_c[:bpad, :])
                else:
                    cw = None
                state[sl] = (lgas, cw)

        def emit_C(sl):
            """Phase C + D for slot sl, one chunk per yield."""
            bpad = bpads[sl]
            lgas, cw = state[sl]
            img_ps = ipsum.tile([128, BPX], F32, tag=f"img{sl % 2}",
                                name=f"img{sl}")
            for c0 in range(0, bpad, CPB):
                blocks = list(range(c0, min(c0 + CPB, bpad)))
                cwid = len(blocks) * BPX
                ps_s = spsum.tile([128, CW], F32,
                                  tag=f"scan{(c0 // CPB) % 2}",
                                  name=f"scan{sl}_{c0}")
                for j, b in enumerate(blocks):
                    sseg = ps_s[:, j * BPX:(j + 1) * BPX]
                    nc.tensor.matmul(sseg, u128[:], lgas[b],
                                     start=True, stop=(b == 0))
                    if b > 0:
                        nc.tensor.matmul(sseg,
                                         eb[:bpad, 128 * b:128 * (b + 1)],
                                         cw[:bpad, :],
                                         start=False, stop=True)
                exT = work.tile([128, CW], F32R, tag="exT")
                nc.scalar.activation(exT[:, :cwid], ps_s[:, :cwid], AF.Exp)
                for j, b in enumerate(blocks):
                    k = koff[sl] + b
                    nc.tensor.matmul(img_ps[:3, :], dcw[:, 3 * k:3 * k + 3],
                                     exT[:, j * BPX:(j + 1) * BPX],
                                     start=(b == 0), stop=(b == bpad - 1))
                yield
            ob = outp.tile([128, BPX], F32, tag="ob")
            nc.vector.tensor_copy(ob[:3, :], img_ps[:3, :])
            nc.sync.dma_start(img_d[:, sl * BPX:(sl + 1) * BPX], ob[:3, :])

        # interleaved emission: phase C of slot sl-1 between phase A
        # chunks of slot sl (in-order engine streams get ready work from
        # the other stream while each chunk's serial chain progresses).
        state = {}
        prev_c = None
        pend = None
        for sl in range(NSLOT):
            for c0 in range(0, bpads[sl], CPB):
                p = emit_A_front(sl, c0)
                if pend is not None:
                    emit_A_back(pend)
                pend = p
                if prev_c is not None:
                    next(prev_c, None)
            # finish the slot's last chunk before its phase C can be built
            emit_A_back(pend)
            pend = None
            if prev_c is not None:
                for _ in prev_c:
                    pass
            prev_c = emit_C(sl)
        for _ in prev_c:
            pass

    saved = bacc.get_activation_tables
    bacc.get_activation_tables = _only_full_act_set
    try:
        nc.compile()
    finally:
        bacc.get_activation_tables = saved
    return nc


# ------------------------------------------------------------------ driver
def kernel(context_pose, target_poses, target_intrinsics, means1, means2,
           cov1, cov2, sh1, sh2, op1, op2, background_color,
           image_h, image_w):
    f32 = np.float32
    b, v = np.asarray(target_poses).shape[:2]
    assert b == 1 and v == NCAM and int(image_h) == H and int(image_w) == W

    context_pose = np.asarray(context_pose, f32)
    target_poses = np.asarray(target_poses, f32)
    target_intrinsics = np.asarray(target_intrinsics, f32)
    bg = np.asarray(background_color, f32)

    try:
        inv_base = np.linalg.inv(
            context_pose[0].astype(np.float64)).astype(f32)
    except np.linalg.LinAlgError:
        inv_base = np.linalg.pinv(
            context_pose[0].astype(np.float64)).astype(f32)
    d_sh = np.asarray(sh1).shape[-1]
    means = np.stack([np.asarray(means1, f32), np.asarray(means2, f32)],
                     1).reshape(-1, 3)
    covs = np.stack([np.asarray(cov1, f32), np.asarray(cov2, f32)],
                    1).reshape(-1, 3, 3)
    shs = np.stack([np.asarray(sh1, f32), np.asarray(sh2, f32)],
                   1).reshape(-1, 3, d_sh)
    ops = np.stack([np.asarray(op1, f32), np.asarray(op2, f32)],
                   1).reshape(-1)
    assert means.shape[0] == G

    row_scale = np.array([1.0 / W, 1.0 / H, 1.0], f32)[:, None]

    cams = []
    for cam in range(NCAM):
        extr = inv_base @ target_poses[0, cam]
        Kn = target_intrinsics[0, cam] * row_scale
        K = np.array([[Kn[0, 0] * W, 0, Kn[0, 2] * W],
                      [0, Kn[1, 1] * H, Kn[1, 2] * H],
                      [0, 0, 1]], f32)
        cams.append(_prep_camera(extr, K, bg, means, covs, shs, ops))
    general = not all(c["psd"] for c in cams)

    # cull per rect, then group the 64 rects by survivor count into NSLOT
    # groups of 8 (one per core): slot j runs the j-th largest group, so
    # padding is per-group, not global max.
    pairs = []
    for p in range(NPAIR):
        cam, rest = divmod(p, NBAND * NXS)
        band, xh = divmod(rest, NXS)
        idx, dc, c0 = _cull_rect(cams[cam], band, xh, bg)
        pairs.append((cam, band, xh, idx, dc, c0))
    order = sorted(range(NPAIR), key=lambda p: -len(pairs[p][3]))
    assign = [[order[g * 8 + i] for i in range(8)] for g in range(NSLOT)]
    # smallest group first: phase A of the first slot has no phase C to
    # overlap with, so keep that un-overlapped head short
    assign = assign[-1:] + assign[:-1]
    bpads = tuple(max(1, -(-max(len(pairs[p][3]) for p in grp) // 128))
                  for grp in assign)

    key = (bool(general), bpads)
    if key not in _NC_CACHE:
        _NC_CACHE[key] = _build_nc(general, bpads)
    nc = _NC_CACHE[key]
    mb = max(bpads)
    koff = [sum(bpads[:i]) for i in range(NSLOT)]
    NBLK = sum(bpads)

    # shared constants
    f16 = np.float16
    bx = np.arange(XW, dtype=f32) + 0.5
    px_x = np.tile(bx, BAND_ROWS)
    px_r = np.repeat(np.arange(BAND_ROWS), XW)
    x2 = (px_x * px_x).astype(f32)
    x2h = x2.astype(f16)                       # exact 2-way split (14 bits)
    x2l = (x2 - x2h.astype(f32)).astype(f16)
    basis = np.zeros((NBAS, BPX), f16)
    basis[0] = x2h
    basis[1] = x2l
    basis[2] = x2h
    basis[3] = x2l
    basis[4] = x2h
    for rr in range(BAND_ROWS):
        xr = (px_x * (px_r == rr)).astype(f16)     # x exact in fp16 (7 bits)
        onr = (1.0 * (px_r == rr)).astype(f16)
        for s in range(3):
            basis[5 + 3 * rr + s] = xr
            basis[5 + 3 * BAND_ROWS + 3 * rr + s] = onr
    u128 = np.triu(np.ones((128, 128), f32))                 # k <= j
    st = np.zeros((128, mb * mb), f32)                       # j > b staircase
    for b_ in range(mb):
        st[:, mb * b_ + b_ + 1:mb * (b_ + 1)] = 1.0
    ebm = np.zeros((mb, mb * 128), f32)                      # carry selector
    for b_ in range(mb):
        ebm[b_, b_ * 128:(b_ + 1) * 128] = 1.0

    in_maps = []
    for core in range(8):
        coefs = np.zeros((NBAS, NBLK * 128), np.float16)
        for rr in range(BAND_ROWS):          # pad columns: D = PAD_C, masked
            coefs[5 + 3 * BAND_ROWS + 3 * rr, :] = PAD_C
        dcv = np.zeros((128, NBLK * 3), f32)
        gsv = np.zeros((128, NBLK), f32)
        for slot in range(NSLOT):
            bpad = bpads[slot]
            cam, band, xh, idx, dc, c0 = pairs[assign[slot][core]]
            cp = cams[cam]
            n = len(idx)
            cfull = _coef_block(cp, idx, band, xh)   # [NBAS, n]
            nlg = -cp["logop"][idx]
            for b_ in range(bpad):
                kf = koff[slot] + b_
                lo, hi = b_ * 128, min(n, (b_ + 1) * 128)
                cnt = max(0, hi - lo)
                if cnt > 0:
                    coefs[:, kf * 128:kf * 128 + cnt] = cfull[:, lo:hi]
                    dcv[:cnt, kf * 3:kf * 3 + 3] = dc[lo:hi]
                    gsv[:cnt, kf] = nlg[lo:hi]
        in_maps.append({"coef": coefs, "basis": basis, "dcw": dcv,
                        "u128": u128, "st": st, "eb": ebm, "gs": gsv})

    trace = os.environ.get("SPLAT_TRACE", "0") == "1"
    res = run_bass_kernel_spmd(nc, in_maps, core_ids=list(range(8)),
                               trace=trace,
                               trace_cores=list(range(8)) if trace else None)
    global _LAST_EXEC_NS, _LAST_RESULTS
    _LAST_EXEC_NS = res.exec_time_ns
    _LAST_RESULTS = res

    out = np.zeros((1, NCAM, 3, H, W), f32)
    for core in range(8):
        img = res.results[core]["img"]
        for slot in range(NSLOT):
            cam, band, xh, idx, dc, c0 = pairs[assign[slot][core]]
            piece = img[:, slot * BPX:(slot + 1) * BPX].reshape(
                3, BAND_ROWS, XW)
            out[0, cam, :, band * BAND_ROWS:(band + 1) * BAND_ROWS,
                xh * XW:(xh + 1) * XW] = piece + c0[:, None, None]
    return out


# revision 38
# speedup vs baseline: 1.0221x; 1.0221x over previous
"""Trainium2 Bass kernel for DecoderSplattingCUDA (EWA Gaussian splatting).

Contract: kernel(**inputs) takes the FULL inputs of reference.setup_inputs()
and returns the FULL [b, v, 3, H, W] image, computed on 8 NeuronCores.

Layout: gaussians on partitions (depth sorted), pixels on the free axis.
The image is split into 128 (camera, 8-row band, x-quarter) rects of
8x32 = 256 px, striped across the 8 cores (16 slots per core) by survivor
count for load balance.  Per rect the host culls gaussians that can reach
alpha >= 1/255 inside the rect (conservative in both y and x, so results
stay exact) and pads survivors to BPAD blocks of 128.  Blocks are
processed in chunks of 2 ([128 g, 512 px] elementwise ops) to amortize
per-instruction access bubbles.

Per chunk:
  D    = coef^T @ basis        (one fp16 matmul per block into a PSUM
                                half; coefficients are host-precomputed
                                3-way fp16 splits of the quadratic
                                A*x^2 + B_r*x + C_r so every product is
                                exact in the f32 accumulate -- fp32r
                                would round inputs to ~11 bits, which the
                                completed-square cancellation amplifies)
  a0   = exp(-D)               (scalar act, straight from PSUM)
  c1   = min(a0, 0.99)         (gpsimd; == the max(D, -ln.99) clamp)
  am   = (a0 >= 1/255) * c1    (one fused vector scalar_tensor_tensor)
  lga  = ln(1 - am)            (scalar act, f32r out)
Depth-ordered transmittance T_g = exp(cumsum lga) is a triangular-ones
fp32r matmul per block (1 cycle/row); carries across blocks come from a
staircase matmul accumulated over the rect's blocks and broadcast back
with a selector-row matmul.  f32r is safe here: its input rounding is
relative to log-T, so T keeps ~2^-11 relative accuracy.  The composite
uses summation by parts: img = c_0 + sum_g (c_{g+1}-c_g) T_g with
c_G := background, so the color matmul contracts T directly.

Emission interleaves phase C of slot sl-1 between the phase-A chunks of
slot sl, so each in-order engine sequencer has ready work from the other
stream while a chunk's serial PE->ACT->Pool->DVE->ACT chain progresses.
"""
import os
import sys

sys.path.insert(0, "/opt/trn_rl_repo/concourse")

from contextlib import ExitStack

import numpy as np

import concourse.bacc as bacc
import concourse.tile as tile
from concourse import mybir
from concourse.bass_utils import run_bass_kernel_spmd
from concourse.hw_specs import get_activation_tables

F32 = mybir.dt.float32
F32R = mybir.dt.float32r
F16 = mybir.dt.float16
AF = mybir.ActivationFunctionType
ALU = mybir.AluOpType

C0 = 0.28209479177387814
C1 = 0.4886025119029199
NEAR, FAR = 0.1, 1000.0

H = W = 128
G = 2048               # gaussians per camera (2 * 32 * 32)
NCAM = 2
BAND_ROWS = 8          # image rows per band
XW = 32                # columns per x-slice
NBAND = H // BAND_ROWS          # bands per camera (16)
NXS = W // XW                   # x-halves (2)
NPAIR = NCAM * NBAND * NXS      # rects (64)
NSLOT = NPAIR // 8              # rects per core (8)
BPX = BAND_ROWS * XW            # pixels per rect (512)
# D-matmul contraction rows: fp16 3-way-split coefficients so products are
# exact in the f32 PSUM accumulate (fp32r would round inputs to ~11 bits,
# which the completed-square cancellation amplifies).
# rows 0-4: A splits x {x2h, x2l} (A2*x2l dropped, ~2^-33 relative)
# rows 5+3r+s: B_r split s against basis x*1_r
# rows 29+3r+s: C_r split s against basis 1_r
NBAS = 5 + 3 * BAND_ROWS + 3 * BAND_ROWS   # 53

LN99 = float(np.float32(-np.log(np.float32(0.99))))     # 0.01005034
LN255 = float(np.float32(np.log(np.float32(255.0))))    # 5.5412636
INV255 = float(np.float32(1.0) / np.float32(255.0))     # 0.00392157
NEG_BIG = -200.0
PAD_C = 1000.0          # padding rows: D = 1000 -> alpha = 0

_NC_CACHE = {}
_LAST_EXEC_NS = None
_LAST_RESULTS = None


def _only_full_act_set(arch):
    """Steer insert_act_table_loads to the one table set that covers
    Exp+Ln+Copy+Identity (natural_log_exp_and_others), so the kernel pays a
    single ACT table load instead of one per function switch."""
    full = get_activation_tables(arch)
    keep = "natural_log_exp_and_others"
    return {name: (fns if name == keep else set()) for name, fns in full.items()}


# ---------------------------------------------------------------- host prep
def _prep_camera(extr, K, bg, means, cov, sh, op):
    """Mirror of reference._render_one's per-gaussian math (numpy f32).
    Returns depth-sorted per-gaussian arrays."""
    f32 = np.float32
    extr = extr.astype(f32)
    try:
        w2c = np.linalg.inv(extr.astype(np.float64)).astype(f32)
    except np.linalg.LinAlgError:
        w2c = np.linalg.pinv(extr.astype(np.float64)).astype(f32)
    R, t = w2c[:3, :3], w2c[:3, 3]
    p = means @ R.T + t
    x, y, z = p[:, 0], p[:, 1], p[:, 2]
    zc = np.maximum(z, f32(1e-6))
    fx, fy = K[0, 0], K[1, 1]
    cx, cy = K[0, 2], K[1, 2]
    u = fx * x / zc + cx
    v = fy * y / zc + cy
    cov_c = np.einsum("ij,gjk,lk->gil", R, cov, R)
    zero = np.zeros_like(zc)
    J = np.stack([np.stack([fx / zc, zero, -fx * x / (zc * zc)], -1),
                  np.stack([zero, fy / zc, -fy * y / (zc * zc)], -1)], -2)
    cov2d = np.einsum("gij,gjk,glk->gil", J, cov_c, J)
    a = cov2d[:, 0, 0] + f32(0.3)
    bb = cov2d[:, 0, 1]
    c = cov2d[:, 1, 1] + f32(0.3)
    det = np.maximum(a * c - bb * bb, f32(1e-12))
    ia, ib, ic = c / det, -bb / det, a / det
    # SH degree-1 -> RGB
    d = means - extr[:3, 3]
    d = d / np.linalg.norm(d, axis=-1, keepdims=True)
    col = C0 * sh[:, :, 0]
    if sh.shape[-1] >= 4:
        col = (col - C1 * d[:, 1:2] * sh[:, :, 1]
               + C1 * d[:, 2:3] * sh[:, :, 2]
               - C1 * d[:, 0:1] * sh[:, :, 3])
    col = np.maximum(col + f32(0.5), f32(0.0)).astype(f32)  # [G, 3]

    valid = (z > f32(NEAR)) & (z < f32(FAR))
    op_eff = np.where(valid, op, f32(0.0))

    order = np.argsort(z, kind="stable")
    u, v, ia, ib, ic, op_eff, z = (arr[order] for arr in
                                   (u, v, ia, ib, ic, op_eff, z))
    col = col[order]

    # completed square: power = -sa*(gamma*(dx + r*dy))^2 - se*(delta*dy)^2
    psd = bool(np.all(ia > 0))
    with np.errstate(divide="ignore", invalid="ignore"):
        r = np.where(ia != 0, ib / ia, f32(0.0)).astype(f32)
        eta = ic - np.where(ia != 0, ib * ib / ia, f32(0.0))
        gamma = np.sqrt(np.abs(ia) * f32(0.5)).astype(f32)
        delta = np.sqrt(np.abs(eta) * f32(0.5)).astype(f32)
        logop = np.where(op_eff > 0, np.log(np.maximum(op_eff, f32(1e-30))),
                         f32(NEG_BIG))
    logop = np.maximum(logop, f32(NEG_BIG)).astype(f32)
    sa = np.sign(ia).astype(f32)
    sa[sa == 0] = 1.0
    se = np.sign(eta).astype(f32)
    se[se == 0] = 1.0
    psd = psd and bool(np.all(eta > 0))
    return dict(u=u.astype(f32), v=v.astype(f32), r=r, gamma=gamma,
                delta=delta, logop=logop, sa=sa, se=se, col=col,
                psd=psd, psd_g=(ia > 0) & (eta > 0))


def _cull_rect(cp, band, xh, bg):
    """Indices (in sorted order) of gaussians that can reach alpha >= 1/255
    anywhere in the rect; conservative in y and x, so dropped ones are
    exactly zero in the reference too.  Returns (idx, dc[3/kept], c0[3])."""
    f32 = np.float32
    ylo = f32(band * BAND_ROWS + 0.5)
    yhi = f32(band * BAND_ROWS + BAND_ROWS - 0.5)
    xlo = f32(xh * XW + 0.5)
    xhi = f32(xh * XW + XW - 0.5)
    v, u = cp["v"], cp["u"]
    dymin = np.maximum(0.0, np.maximum(ylo - v, v - yhi)).astype(f32)
    budget = cp["logop"] + f32(LN255 + 0.01)
    yterm = (cp["delta"] * dymin) ** 2
    keep = yterm <= budget
    # x-reach: s = x + r*dy - u is zero at x = u - r*dy; over the band's dy
    # range the zero sweeps an interval; distance from the rect to it bounds
    # |s| from below (conservative: continuous dy range contains row centers)
    sh = np.stack([cp["r"] * (ylo - v), cp["r"] * (yhi - v)])
    c_lo = u - sh.max(0)
    c_hi = u - sh.min(0)
    dxmin = np.maximum(0.0, np.maximum(xlo - c_hi, c_lo - xhi)).astype(f32)
    keep &= ((cp["gamma"] * dxmin) ** 2 + yterm) <= budget
    keep |= ~cp["psd_g"]     # non-PSD conics: never cull
    idx = np.nonzero(keep)[0]
    col = cp["col"][idx]
    n = len(idx)
    dc = np.zeros((n, 3), f32)
    if n:
        dc[:-1] = col[1:] - col[:-1]
        dc[-1] = bg - col[-1]
        c0 = col[0].copy()
    else:
        c0 = bg.astype(f32).copy()
    return idx, dc, c0


def _split3(v):
    """f32 -> three fp16 parts summing to ~33-bit precision."""
    f32, f16 = np.float32, np.float16
    v0 = v.astype(f16)
    r1 = (v - v0.astype(f32)).astype(f32)
    v1 = r1.astype(f16)
    v2 = (r1 - v1.astype(f32)).astype(f16)
    return v0, v1, v2


def _coef_block(cp, idx, band, xh):
    """Host-side D-matmul coefficients [NBAS, n] fp16 (3-way split) for one
    rect's survivors: D = A*x^2 + B_r*x + C_r per band row r, x local+0.5."""
    f32 = np.float32
    u_ = cp["u"][idx]
    r_ = cp["r"][idx]
    g_ = cp["gamma"][idx]
    v_ = cp["v"][idx]
    d_ = cp["delta"][idx]
    lo_ = cp["logop"][idx]
    sa = cp["sa"][idx]
    se = cp["se"][idx]
    dy = (np.arange(BAND_ROWS, dtype=f32) + band * BAND_ROWS
          + 0.5)[None, :] - v_[:, None]                     # [n, 8]
    A = (sa * g_ * g_).astype(f32)
    E = (r_[:, None] * dy - u_[:, None] + f32(xh * XW)).astype(f32)
    B = np.clip((2.0 * A[:, None] * E), -6e4, 6e4).astype(f32)
    C = np.clip((A[:, None] * E * E + (se * d_ * d_)[:, None] * dy * dy
                 - lo_[:, None]), -6e4, 6e4).astype(f32)
    n = len(idx)
    A0, A1, A2 = _split3(np.clip(A, -6e4, 6e4))
    B0, B1, B2 = _split3(B)    # [n, 8] each
    Cs = _split3(C)
    coef = np.zeros((NBAS, n), np.float16)
    coef[0] = A0
    coef[1] = A0
    coef[2] = A1
    coef[3] = A1
    coef[4] = A2
    for rr in range(BAND_ROWS):
        for s, Bs in enumerate((B0, B1, B2)):
            coef[5 + 3 * rr + s] = Bs[:, rr]
        for s in range(3):
            coef[5 + 3 * BAND_ROWS + 3 * rr + s] = Cs[s][:, rr]
    return coef


# ------------------------------------------------------------- bass program
def _build_nc(general: bool, bpads: tuple):
    nc = bacc.Bacc(None, target_bir_lowering=False)

    NBLK = sum(bpads)
    mb = max(bpads)
    koff = [sum(bpads[:i]) for i in range(NSLOT)]
    coef_d = nc.dram_tensor("coef", [NBAS, NBLK * 128], F16,
                            kind="ExternalInput")
    basis_d = nc.dram_tensor("basis", [NBAS, BPX], F16, kind="ExternalInput")
    dc_d = nc.dram_tensor("dcw", [128, NBLK * 3], F32R, kind="ExternalInput")
    u128_d = nc.dram_tensor("u128", [128, 128], F32R, kind="ExternalInput")
    st_d = nc.dram_tensor("st", [128, mb * mb], F32R, kind="ExternalInput")
    eb_d = nc.dram_tensor("eb", [mb, mb * 128], F32R, kind="ExternalInput")
    gs_d = nc.dram_tensor("gs", [128, NBLK], F32, kind="ExternalInput")
    img_d = nc.dram_tensor("img", [3, NSLOT * BPX], F32, kind="ExternalOutput")

    CPB = 512 // BPX        # blocks per elementwise chunk (PSUM: 1 bank)
    CW = CPB * BPX          # chunk width (512)

    with tile.TileContext(nc) as tc, ExitStack() as ctx:
        consts = ctx.enter_context(tc.tile_pool(name="consts", bufs=1))
        work = ctx.enter_context(tc.tile_pool(name="work", bufs=4))
        lgap = ctx.enter_context(
            tc.tile_pool(name="lgap", bufs=3 * (-(-mb // CPB))))
        carp = ctx.enter_context(tc.tile_pool(name="carp", bufs=2))
        outp = ctx.enter_context(tc.tile_pool(name="outp", bufs=2))
        dpsum = ctx.enter_context(tc.tile_pool(name="dpsum", bufs=1,
                                               space="PSUM"))
        cpsum = ctx.enter_context(tc.tile_pool(name="cpsum", bufs=1,
                                               space="PSUM"))
        spsum = ctx.enter_context(tc.tile_pool(name="spsum", bufs=1,
                                               space="PSUM"))
        ipsum = ctx.enter_context(tc.tile_pool(name="ipsum", bufs=1,
                                               space="PSUM"))

        # coef is large; split its load per-slot so the first chunks only
        # wait on their own slice, and spread loads over separate queues.
        coefs_t = [consts.tile([NBAS, bpads[s] * 128], F16, name=f"coef{s}")
                   for s in range(NSLOT)]
        basis = consts.tile([NBAS, BPX], F16)
        dcw = consts.tile([128, NBLK * 3], F32R)
        u128 = consts.tile([128, 128], F32R)
        st = consts.tile([128, mb * mb], F32R)
        eb = consts.tile([mb, mb * 128], F32R)
        gs = consts.tile([128, NBLK], F32)
        nc.sync.dma_start(basis[:], basis_d[:])
        nc.gpsimd.dma_start(u128[:], u128_d[:])
        nc.gpsimd.dma_start(dcw[:], dc_d[:])
        nc.gpsimd.dma_start(st[:], st_d[:])
        nc.gpsimd.dma_start(eb[:], eb_d[:])
        if general:
            nc.gpsimd.dma_start(gs[:], gs_d[:])
        for s in range(NSLOT):
            nc.sync.dma_start(
                coefs_t[s][:],
                coef_d[:, koff[s] * 128:(koff[s] + bpads[s]) * 128])

        def coef_ap(k):
            sl = max(s for s in range(NSLOT) if koff[s] <= k)
            b = k - koff[sl]
            return coefs_t[sl][:, b * 128:(b + 1) * 128]

        slot_ps_c = {}
        slot_lgas = {}

        def emit_A_front(sl, c0):
            """D matmuls + Exp for one chunk; returns pend for the back."""
            bpad = bpads[sl]
            if sl not in slot_ps_c:
                slot_ps_c[sl] = (cpsum.tile([128, BPX], F32,
                                            tag="ps_c",
                                            name=f"ps_c{sl}")
                                 if bpad > 1 else None)
                slot_lgas[sl] = []
            blocks = list(range(c0, min(c0 + CPB, bpad)))
            cwid = len(blocks) * BPX
            ps_d = dpsum.tile([128, CW], F32, tag=f"ps_d{(c0 // CPB) % 3}",
                              name=f"ps_d{sl}_{c0}")
            for j, b in enumerate(blocks):
                k = koff[sl] + b
                nc.tensor.matmul(ps_d[:, j * BPX:(j + 1) * BPX],
                                 coef_ap(k),
                                 basis[:], start=True, stop=True)
            # a0 = exp(-D) straight from PSUM; the 0.99 clamp moves to a
            # Pool min (max(D, ln99) pre-exp == min(a0, 0.99) post-exp)
            # and the 1/255 cull is one fused DVE stt on a0 itself.
            a0 = work.tile([128, CW], F32, tag="a0")
            if general:
                # guard: non-PSD D can be hugely negative -> exp inf;
                # clamp D first (slower, correctness-only path)
                Dc = work.tile([128, CW], F32, tag="Dc")
                nc.vector.tensor_scalar(Dc[:, :cwid], ps_d[:, :cwid],
                                        LN99, None, ALU.max)
                nc.scalar.activation(a0[:, :cwid], Dc[:, :cwid],
                                     AF.Exp, scale=-1.0)
            else:
                nc.scalar.activation(a0[:, :cwid], ps_d[:, :cwid],
                                     AF.Exp, scale=-1.0)
            return (sl, blocks, cwid, ps_d, a0)

        def emit_A_back(pend):
            """min/stt/Ln/stair for a front-stage chunk; finishes the slot's
            phase B when it was the slot's last chunk."""
            sl, blocks, cwid, ps_d, a0 = pend
            bpad = bpads[sl]
            ps_c = slot_ps_c[sl]
            lgas = slot_lgas[sl]
            c1 = work.tile([128, CW], F32, tag="c1")
            nc.gpsimd.tensor_scalar(c1[:, :cwid], a0[:, :cwid], 0.99,
                                    None, ALU.min)
            am = work.tile([128, CW], F32, tag="am")
            nc.vector.scalar_tensor_tensor(am[:, :cwid], a0[:, :cwid],
                                           INV255, c1[:, :cwid],
                                           ALU.is_ge, ALU.mult)
            if general:
                am2 = work.tile([128, CW], F32, tag="am2")
                for j, b in enumerate(blocks):
                    k = koff[sl] + b
                    nc.vector.scalar_tensor_tensor(
                        am2[:, j * BPX:(j + 1) * BPX],
                        ps_d[:, j * BPX:(j + 1) * BPX],
                        gs[:, k:k + 1], am[:, j * BPX:(j + 1) * BPX],
                        ALU.is_ge, ALU.mult)
                am = am2
            lga = lgap.tile([128, CW], F32R, tag="lga")
            nc.scalar.activation(lga[:, :cwid], am[:, :cwid], AF.Ln,
                                 scale=-1.0, bias=1.0)
            for j, b in enumerate(blocks):
                lgas.append(lga[:, j * BPX:(j + 1) * BPX])
                if bpad > 1 and b < bpad - 1:
                    nc.tensor.matmul(ps_c[:bpad, :],
                                     st[:, mb * b:mb * b + bpad],
                                     lga[:, j * BPX:(j + 1) * BPX],
                                     start=(b == 0),
                                     stop=(b == bpad - 2))
            if blocks[-1] == bpad - 1:
                # phase B: carries to SBUF (f32, no compensation needed)
                if bpad > 1:
                    cw = carp.tile([128, BPX], F32R, tag="cw")
                    nc.vector.tensor_copy(cw[:bpad, :], ps_c[:bpad, :])
                else:
                    cw = None
                state[sl] = (lgas, cw)

        def emit_C(sl):
            """Phase C + D for slot sl, one chunk per yield."""
            bpad = bpads[sl]
            lgas, cw = state[sl]
            img_ps = ipsum.tile([128, BPX], F32, tag=f"img{sl % 2}",
                                name=f"img{sl}")
            for c0 in range(0, bpad, CPB):
                blocks = list(range(c0, min(c0 + CPB, bpad)))
                cwid = len(blocks) * BPX
                ps_s = spsum.tile([128, CW], F32,
                                  tag=f"scan{(c0 // CPB) % 2}",
                                  name=f"scan{sl}_{c0}")
                for j, b in enumerate(blocks):
                    sseg = ps_s[:, j * BPX:(j + 1) * BPX]
                    nc.tensor.matmul(sseg, u128[:], lgas[b],
                                     start=True, stop=(b == 0))
                    if b > 0:
                        nc.tensor.matmul(sseg,
                                         eb[:bpad, 128 * b:128 * (b + 1)],
                                         cw[:bpad, :],
                                         start=False, stop=True)
                exT = work.tile([128, CW], F32R, tag="exT")
                nc.scalar.activation(exT[:, :cwid], ps_s[:, :cwid], AF.Exp)
                for j, b in enumerate(blocks):
                    k = koff[sl] + b
                    nc.tensor.matmul(img_ps[:3, :], dcw[:, 3 * k:3 * k + 3],
                                     exT[:, j * BPX:(j + 1) * BPX],
                                     start=(b == 0), stop=(b == bpad - 1))
                yield
            ob = outp.tile([128, BPX], F32, tag="ob")
            nc.vector.tensor_copy(ob[:3, :], img_ps[:3, :])
            nc.sync.dma_start(img_d[:, sl * BPX:(sl + 1) * BPX], ob[:3, :])

        # interleaved emission: phase C of slot sl-1 between phase A
        # chunks of slot sl (in-order engine streams get ready work from
        # the other stream while each chunk's serial chain progresses).
        state = {}
        prev_c = None
        pend = None
        for sl in range(NSLOT):
            for c0 in range(0, bpads[sl], CPB):
                p = emit_A_front(sl, c0)
                if pend is not None:
                    emit_A_back(pend)
                pend = p
                if prev_c is not None:
                    next(prev_c, None)
            # finish the slot's last chunk before its phase C can be built
            emit_A_back(pend)
            pend = None
            if prev_c is not None:
                for _ in prev_c:
                    pass
            prev_c = emit_C(sl)
        for _ in prev_c:
            pass

    saved = bacc.get_activation_tables
    bacc.get_activation_tables = _only_full_act_set
    try:
        nc.compile()
    finally:
        bacc.get_activation_tables = saved
    return nc


# ------------------------------------------------------------------ driver
def kernel(context_pose, target_poses, target_intrinsics, means1, means2,
           cov1, cov2, sh1, sh2, op1, op2, background_color,
           image_h, image_w):
    f32 = np.float32
    b, v = np.asarray(target_poses).shape[:2]
    assert b == 1 and v == NCAM and int(image_h) == H and int(image_w) == W

    context_pose = np.asarray(context_pose, f32)
    target_poses = np.asarray(target_poses, f32)
    target_intrinsics = np.asarray(target_intrinsics, f32)
    bg = np.asarray(background_color, f32)

    try:
        inv_base = np.linalg.inv(
            context_pose[0].astype(np.float64)).astype(f32)
    except np.linalg.LinAlgError:
        inv_base = np.linalg.pinv(
            context_pose[0].astype(np.float64)).astype(f32)
    d_sh = np.asarray(sh1).shape[-1]
    means = np.stack([np.asarray(means1, f32), np.asarray(means2, f32)],
                     1).reshape(-1, 3)
    covs = np.stack([np.asarray(cov1, f32), np.asarray(cov2, f32)],
                    1).reshape(-1, 3, 3)
    shs = np.stack([np.asarray(sh1, f32), np.asarray(sh2, f32)],
                   1).reshape(-1, 3, d_sh)
    ops = np.stack([np.asarray(op1, f32), np.asarray(op2, f32)],
                   1).reshape(-1)
    assert means.shape[0] == G

    row_scale = np.array([1.0 / W, 1.0 / H, 1.0], f32)[:, None]

    cams = []
    for cam in range(NCAM):
        extr = inv_base @ target_poses[0, cam]
        Kn = target_intrinsics[0, cam] * row_scale
        K = np.array([[Kn[0, 0] * W, 0, Kn[0, 2] * W],
                      [0, Kn[1, 1] * H, Kn[1, 2] * H],
                      [0, 0, 1]], f32)
        cams.append(_prep_camera(extr, K, bg, means, covs, shs, ops))
    general = not all(c["psd"] for c in cams)

    # cull per rect, then group the 64 rects by survivor count into NSLOT
    # groups of 8 (one per core): slot j runs the j-th largest group, so
    # padding is per-group, not global max.
    pairs = []
    for p in range(NPAIR):
        cam, rest = divmod(p, NBAND * NXS)
        band, xh = divmod(rest, NXS)
        idx, dc, c0 = _cull_rect(cams[cam], band, xh, bg)
        pairs.append((cam, band, xh, idx, dc, c0))
    order = sorted(range(NPAIR), key=lambda p: -len(pairs[p][3]))
    assign = [[order[g * 8 + i] for i in range(8)] for g in range(NSLOT)]
    bpads = tuple(max(1, -(-max(len(pairs[p][3]) for p in grp) // 128))
                  for grp in assign)

    key = (bool(general), bpads)
    if key not in _NC_CACHE:
        _NC_CACHE[key] = _build_nc(general, bpads)
    nc = _NC_CACHE[key]
    mb = max(bpads)
    koff = [sum(bpads[:i]) for i in range(NSLOT)]
    NBLK = sum(bpads)

    # shared constants
    f16 = np.float16
    bx = np.arange(XW, dtype=f32) + 0.5
    px_x = np.tile(bx, BAND_ROWS)
    px_r = np.repeat(np.arange(BAND_ROWS), XW)
    x2 = (px_x * px_x).astype(f32)
    x2h = x2.astype(f16)                       # exact 2-way split (14 bits)
    x2l = (x2 - x2h.astype(f32)).astype(f16)
    basis = np.zeros((NBAS, BPX), f16)
    basis[0] = x2h
    basis[1] = x2l
    basis[2] = x2h
    basis[3] = x2l
    basis[4] = x2h
    for rr in range(BAND_ROWS):
        xr = (px_x * (px_r == rr)).astype(f16)     # x exact in fp16 (7 bits)
        onr = (1.0 * (px_r == rr)).astype(f16)
        for s in range(3):
            basis[5 + 3 * rr + s] = xr
            basis[5 + 3 * BAND_ROWS + 3 * rr + s] = onr
    u128 = np.triu(np.ones((128, 128), f32))                 # k <= j
    st = np.zeros((128, mb * mb), f32)                       # j > b staircase
    for b_ in range(mb):
        st[:, mb * b_ + b_ + 1:mb * (b_ + 1)] = 1.0
    ebm = np.zeros((mb, mb * 128), f32)                      # carry selector
    for b_ in range(mb):
        ebm[b_, b_ * 128:(b_ + 1) * 128] = 1.0

    in_maps = []
    for core in range(8):
        coefs = np.zeros((NBAS, NBLK * 128), np.float16)
        for rr in range(BAND_ROWS):          # pad columns: D = PAD_C, masked
            coefs[5 + 3 * BAND_ROWS + 3 * rr, :] = PAD_C
        dcv = np.zeros((128, NBLK * 3), f32)
        gsv = np.zeros((128, NBLK), f32)
        for slot in range(NSLOT):
            bpad = bpads[slot]
            cam, band, xh, idx, dc, c0 = pairs[assign[slot][core]]
            cp = cams[cam]
            n = len(idx)
            cfull = _coef_block(cp, idx, band, xh)   # [NBAS, n]
            nlg = -cp["logop"][idx]
            for b_ in range(bpad):
                kf = koff[slot] + b_
                lo, hi = b_ * 128, min(n, (b_ + 1) * 128)
                cnt = max(0, hi - lo)
                if cnt > 0:
                    coefs[:, kf * 128:kf * 128 + cnt] = cfull[:, lo:hi]
                    dcv[:cnt, kf * 3:kf * 3 + 3] = dc[lo:hi]
                    gsv[:cnt, kf] = nlg[lo:hi]
        in_maps.append({"coef": coefs, "basis": basis, "dcw": dcv,
                        "u128": u128, "st": st, "eb": ebm, "gs": gsv})

    trace = os.environ.get("SPLAT_TRACE", "0") == "1"
    res = run_bass_kernel_spmd(nc, in_maps, core_ids=list(range(8)),
                               trace=trace,
                               trace_cores=list(range(8)) if trace else None)
    global _LAST_EXEC_NS, _LAST_RESULTS
    _LAST_EXEC_NS = res.exec_time_ns
    _LAST_RESULTS = res

    out = np.zeros((1, NCAM, 3, H, W), f32)
    for core in range(8):
        img = res.results[core]["img"]
        for slot in range(NSLOT):
            cam, band, xh, idx, dc, c0 = pairs[assign[slot][core]]
            piece = img[:, slot * BPX:(slot + 1) * BPX].reshape(
                3, BAND_ROWS, XW)
            out[0, cam, :, band * BAND_ROWS:(band + 1) * BAND_ROWS,
                xh * XW:(xh + 1) * XW] = piece + c0[:, None, None]
    return out


# revision 39
# speedup vs baseline: 1.0566x; 1.0338x over previous
"""Trainium2 Bass kernel for DecoderSplattingCUDA (EWA Gaussian splatting).

Contract: kernel(**inputs) takes the FULL inputs of reference.setup_inputs()
and returns the FULL [b, v, 3, H, W] image, computed on 8 NeuronCores.

Layout: gaussians on partitions (depth sorted), pixels on the free axis.
The image is split into 128 (camera, 8-row band, x-quarter) rects of
8x32 = 256 px, striped across the 8 cores (16 slots per core) by survivor
count for load balance.  Per rect the host culls gaussians that can reach
alpha >= 1/255 inside the rect (conservative in both y and x, so results
stay exact) and pads survivors to BPAD blocks of 128.  Blocks are
processed in chunks of 2 ([128 g, 512 px] elementwise ops) to amortize
per-instruction access bubbles.

Per chunk:
  D    = coef^T @ basis        (one fp16 matmul per block into a PSUM
                                half; coefficients are host-precomputed
                                3-way fp16 splits of the quadratic
                                A*x^2 + B_r*x + C_r so every product is
                                exact in the f32 accumulate -- fp32r
                                would round inputs to ~11 bits, which the
                                completed-square cancellation amplifies)
  a0   = exp(-D)               (scalar act, straight from PSUM)
  c1   = min(a0, 0.99)         (gpsimd; == the max(D, -ln.99) clamp)
  am   = (a0 >= 1/255) * c1    (one fused vector scalar_tensor_tensor)
  lga  = ln(1 - am)            (scalar act, f32r out)
Depth-ordered transmittance T_g = exp(cumsum lga) is a triangular-ones
fp32r matmul per block (1 cycle/row); carries across blocks come from a
staircase matmul accumulated over the rect's blocks and broadcast back
with a selector-row matmul.  f32r is safe here: its input rounding is
relative to log-T, so T keeps ~2^-11 relative accuracy.  The composite
uses summation by parts: img = c_0 + sum_g (c_{g+1}-c_g) T_g with
c_G := background, so the color matmul contracts T directly.

Emission interleaves phase C of slot sl-1 between the phase-A chunks of
slot sl, so each in-order engine sequencer has ready work from the other
stream while a chunk's serial PE->ACT->Pool->DVE->ACT chain progresses.
"""
import os
import sys

sys.path.insert(0, "/opt/trn_rl_repo/concourse")

from contextlib import ExitStack

import numpy as np

import concourse.bacc as bacc
import concourse.tile as tile
from concourse import mybir
from concourse.bass_utils import run_bass_kernel_spmd
from concourse.hw_specs import get_activation_tables

F32 = mybir.dt.float32
F32R = mybir.dt.float32r
F16 = mybir.dt.float16
AF = mybir.ActivationFunctionType
ALU = mybir.AluOpType

C0 = 0.28209479177387814
C1 = 0.4886025119029199
NEAR, FAR = 0.1, 1000.0

H = W = 128
G = 2048               # gaussians per camera (2 * 32 * 32)
NCAM = 2
BAND_ROWS = 8          # image rows per band
XW = 32                # columns per x-slice
NBAND = H // BAND_ROWS          # bands per camera (16)
NXS = W // XW                   # x-halves (2)
NPAIR = NCAM * NBAND * NXS      # rects (64)
NSLOT = NPAIR // 8              # rects per core (8)
BPX = BAND_ROWS * XW            # pixels per rect (512)
# D-matmul contraction rows: fp16 3-way-split coefficients so products are
# exact in the f32 PSUM accumulate (fp32r would round inputs to ~11 bits,
# which the completed-square cancellation amplifies).
# rows 0-4: A splits x {x2h, x2l} (A2*x2l dropped, ~2^-33 relative)
# rows 5+3r+s: B_r split s against basis x*1_r
# rows 29+3r+s: C_r split s against basis 1_r
NBAS = 5 + 3 * BAND_ROWS + 3 * BAND_ROWS   # 53

LN99 = float(np.float32(-np.log(np.float32(0.99))))     # 0.01005034
LN255 = float(np.float32(np.log(np.float32(255.0))))    # 5.5412636
INV255 = float(np.float32(1.0) / np.float32(255.0))     # 0.00392157
NEG_BIG = -200.0
PAD_C = 1000.0          # padding rows: D = 1000 -> alpha = 0

_NC_CACHE = {}
_LAST_EXEC_NS = None
_LAST_RESULTS = None


def _only_full_act_set(arch):
    """Steer insert_act_table_loads to the one table set that covers
    Exp+Ln+Copy+Identity (natural_log_exp_and_others), so the kernel pays a
    single ACT table load instead of one per function switch."""
    full = get_activation_tables(arch)
    keep = "natural_log_exp_and_others"
    return {name: (fns if name == keep else set()) for name, fns in full.items()}


# ---------------------------------------------------------------- host prep
def _prep_camera(extr, K, bg, means, cov, sh, op):
    """Mirror of reference._render_one's per-gaussian math (numpy f32).
    Returns depth-sorted per-gaussian arrays."""
    f32 = np.float32
    extr = extr.astype(f32)
    try:
        w2c = np.linalg.inv(extr.astype(np.float64)).astype(f32)
    except np.linalg.LinAlgError:
        w2c = np.linalg.pinv(extr.astype(np.float64)).astype(f32)
    R, t = w2c[:3, :3], w2c[:3, 3]
    p = means @ R.T + t
    x, y, z = p[:, 0], p[:, 1], p[:, 2]
    zc = np.maximum(z, f32(1e-6))
    fx, fy = K[0, 0], K[1, 1]
    cx, cy = K[0, 2], K[1, 2]
    u = fx * x / zc + cx
    v = fy * y / zc + cy
    cov_c = np.einsum("ij,gjk,lk->gil", R, cov, R)
    zero = np.zeros_like(zc)
    J = np.stack([np.stack([fx / zc, zero, -fx * x / (zc * zc)], -1),
                  np.stack([zero, fy / zc, -fy * y / (zc * zc)], -1)], -2)
    cov2d = np.einsum("gij,gjk,glk->gil", J, cov_c, J)
    a = cov2d[:, 0, 0] + f32(0.3)
    bb = cov2d[:, 0, 1]
    c = cov2d[:, 1, 1] + f32(0.3)
    det = np.maximum(a * c - bb * bb, f32(1e-12))
    ia, ib, ic = c / det, -bb / det, a / det
    # SH degree-1 -> RGB
    d = means - extr[:3, 3]
    d = d / np.linalg.norm(d, axis=-1, keepdims=True)
    col = C0 * sh[:, :, 0]
    if sh.shape[-1] >= 4:
        col = (col - C1 * d[:, 1:2] * sh[:, :, 1]
               + C1 * d[:, 2:3] * sh[:, :, 2]
               - C1 * d[:, 0:1] * sh[:, :, 3])
    col = np.maximum(col + f32(0.5), f32(0.0)).astype(f32)  # [G, 3]

    valid = (z > f32(NEAR)) & (z < f32(FAR))
    op_eff = np.where(valid, op, f32(0.0))

    order = np.argsort(z, kind="stable")
    u, v, ia, ib, ic, op_eff, z = (arr[order] for arr in
                                   (u, v, ia, ib, ic, op_eff, z))
    col = col[order]

    # completed square: power = -sa*(gamma*(dx + r*dy))^2 - se*(delta*dy)^2
    psd = bool(np.all(ia > 0))
    with np.errstate(divide="ignore", invalid="ignore"):
        r = np.where(ia != 0, ib / ia, f32(0.0)).astype(f32)
        eta = ic - np.where(ia != 0, ib * ib / ia, f32(0.0))
        gamma = np.sqrt(np.abs(ia) * f32(0.5)).astype(f32)
        delta = np.sqrt(np.abs(eta) * f32(0.5)).astype(f32)
        logop = np.where(op_eff > 0, np.log(np.maximum(op_eff, f32(1e-30))),
                         f32(NEG_BIG))
    logop = np.maximum(logop, f32(NEG_BIG)).astype(f32)
    sa = np.sign(ia).astype(f32)
    sa[sa == 0] = 1.0
    se = np.sign(eta).astype(f32)
    se[se == 0] = 1.0
    psd = psd and bool(np.all(eta > 0))
    return dict(u=u.astype(f32), v=v.astype(f32), r=r, gamma=gamma,
                delta=delta, logop=logop, sa=sa, se=se, col=col,
                psd=psd, psd_g=(ia > 0) & (eta > 0))


def _cull_rect(cp, band, xh, bg):
    """Indices (in sorted order) of gaussians that can reach alpha >= 1/255
    anywhere in the rect; conservative in y and x, so dropped ones are
    exactly zero in the reference too.  Returns (idx, dc[3/kept], c0[3])."""
    f32 = np.float32
    ylo = f32(band * BAND_ROWS + 0.5)
    yhi = f32(band * BAND_ROWS + BAND_ROWS - 0.5)
    xlo = f32(xh * XW + 0.5)
    xhi = f32(xh * XW + XW - 0.5)
    v, u = cp["v"], cp["u"]
    dymin = np.maximum(0.0, np.maximum(ylo - v, v - yhi)).astype(f32)
    budget = cp["logop"] + f32(LN255 + 0.01)
    yterm = (cp["delta"] * dymin) ** 2
    keep = yterm <= budget
    # x-reach: s = x + r*dy - u is zero at x = u - r*dy; over the band's dy
    # range the zero sweeps an interval; distance from the rect to it bounds
    # |s| from below (conservative: continuous dy range contains row centers)
    sh = np.stack([cp["r"] * (ylo - v), cp["r"] * (yhi - v)])
    c_lo = u - sh.max(0)
    c_hi = u - sh.min(0)
    dxmin = np.maximum(0.0, np.maximum(xlo - c_hi, c_lo - xhi)).astype(f32)
    keep &= ((cp["gamma"] * dxmin) ** 2 + yterm) <= budget
    keep |= ~cp["psd_g"]     # non-PSD conics: never cull
    idx = np.nonzero(keep)[0]
    col = cp["col"][idx]
    n = len(idx)
    dc = np.zeros((n, 3), f32)
    if n:
        dc[:-1] = col[1:] - col[:-1]
        dc[-1] = bg - col[-1]
        c0 = col[0].copy()
    else:
        c0 = bg.astype(f32).copy()
    return idx, dc, c0


def _split3(v):
    """f32 -> three fp16 parts summing to ~33-bit precision."""
    f32, f16 = np.float32, np.float16
    v0 = v.astype(f16)
    r1 = (v - v0.astype(f32)).astype(f32)
    v1 = r1.astype(f16)
    v2 = (r1 - v1.astype(f32)).astype(f16)
    return v0, v1, v2


def _coef_block(cp, idx, band, xh):
    """Host-side D-matmul coefficients [NBAS, n] fp16 (3-way split) for one
    rect's survivors: D = A*x^2 + B_r*x + C_r per band row r, x local+0.5."""
    f32 = np.float32
    u_ = cp["u"][idx]
    r_ = cp["r"][idx]
    g_ = cp["gamma"][idx]
    v_ = cp["v"][idx]
    d_ = cp["delta"][idx]
    lo_ = cp["logop"][idx]
    sa = cp["sa"][idx]
    se = cp["se"][idx]
    dy = (np.arange(BAND_ROWS, dtype=f32) + band * BAND_ROWS
          + 0.5)[None, :] - v_[:, None]                     # [n, 8]
    A = (sa * g_ * g_).astype(f32)
    E = (r_[:, None] * dy - u_[:, None] + f32(xh * XW)).astype(f32)
    B = np.clip((2.0 * A[:, None] * E), -6e4, 6e4).astype(f32)
    C = np.clip((A[:, None] * E * E + (se * d_ * d_)[:, None] * dy * dy
                 - lo_[:, None]), -6e4, 6e4).astype(f32)
    n = len(idx)
    A0, A1, A2 = _split3(np.clip(A, -6e4, 6e4))
    B0, B1, B2 = _split3(B)    # [n, 8] each
    Cs = _split3(C)
    coef = np.zeros((NBAS, n), np.float16)
    coef[0] = A0
    coef[1] = A0
    coef[2] = A1
    coef[3] = A1
    coef[4] = A2
    for rr in range(BAND_ROWS):
        for s, Bs in enumerate((B0, B1, B2)):
            coef[5 + 3 * rr + s] = Bs[:, rr]
        for s in range(3):
            coef[5 + 3 * BAND_ROWS + 3 * rr + s] = Cs[s][:, rr]
    return coef


# ------------------------------------------------------------- bass program
def _build_nc(general: bool, bpads: tuple):
    nc = bacc.Bacc(None, target_bir_lowering=False)

    NBLK = sum(bpads)
    mb = max(bpads)
    koff = [sum(bpads[:i]) for i in range(NSLOT)]
    coef_d = nc.dram_tensor("coef", [NBAS, NBLK * 128], F16,
                            kind="ExternalInput")
    basis_d = nc.dram_tensor("basis", [NBAS, BPX], F16, kind="ExternalInput")
    dc_d = nc.dram_tensor("dcw", [128, NBLK * 3], F32R, kind="ExternalInput")
    u128_d = nc.dram_tensor("u128", [128, 128], F32R, kind="ExternalInput")
    st_d = nc.dram_tensor("st", [128, mb * mb], F32R, kind="ExternalInput")
    eb_d = nc.dram_tensor("eb", [mb, mb * 128], F32R, kind="ExternalInput")
    gs_d = nc.dram_tensor("gs", [128, NBLK], F32, kind="ExternalInput")
    img_d = nc.dram_tensor("img", [3, NSLOT * BPX], F32, kind="ExternalOutput")

    CPB = 512 // BPX        # blocks per elementwise chunk (PSUM: 1 bank)
    CW = CPB * BPX          # chunk width (512)

    with tile.TileContext(nc) as tc, ExitStack() as ctx:
        consts = ctx.enter_context(tc.tile_pool(name="consts", bufs=1))
        work = ctx.enter_context(tc.tile_pool(name="work", bufs=4))
        lgap = ctx.enter_context(
            tc.tile_pool(name="lgap", bufs=3 * (-(-mb // CPB))))
        carp = ctx.enter_context(tc.tile_pool(name="carp", bufs=2))
        outp = ctx.enter_context(tc.tile_pool(name="outp", bufs=2))
        dpsum = ctx.enter_context(tc.tile_pool(name="dpsum", bufs=1,
                                               space="PSUM"))
        cpsum = ctx.enter_context(tc.tile_pool(name="cpsum", bufs=1,
                                               space="PSUM"))
        spsum = ctx.enter_context(tc.tile_pool(name="spsum", bufs=1,
                                               space="PSUM"))
        ipsum = ctx.enter_context(tc.tile_pool(name="ipsum", bufs=1,
                                               space="PSUM"))

        # coef is large; split its load per-slot so the first chunks only
        # wait on their own slice, and spread loads over separate queues.
        coefs_t = [consts.tile([NBAS, bpads[s] * 128], F16, name=f"coef{s}")
                   for s in range(NSLOT)]
        basis = consts.tile([NBAS, BPX], F16)
        dcw = consts.tile([128, NBLK * 3], F32R)
        u128 = consts.tile([128, 128], F32R)
        st = consts.tile([128, mb * mb], F32R)
        eb = consts.tile([mb, mb * 128], F32R)
        gs = consts.tile([128, NBLK], F32)
        nc.sync.dma_start(basis[:], basis_d[:])
        nc.gpsimd.dma_start(u128[:], u128_d[:])
        nc.gpsimd.dma_start(dcw[:], dc_d[:])
        nc.gpsimd.dma_start(st[:], st_d[:])
        nc.gpsimd.dma_start(eb[:], eb_d[:])
        if general:
            nc.gpsimd.dma_start(gs[:], gs_d[:])
        for s in range(NSLOT):
            nc.sync.dma_start(
                coefs_t[s][:],
                coef_d[:, koff[s] * 128:(koff[s] + bpads[s]) * 128])

        def coef_ap(k):
            sl = max(s for s in range(NSLOT) if koff[s] <= k)
            b = k - koff[sl]
            return coefs_t[sl][:, b * 128:(b + 1) * 128]

        slot_ps_c = {}
        slot_lgas = {}

        def emit_A_front(sl, c0):
            """D matmuls + Exp for one chunk; returns pend for the back."""
            bpad = bpads[sl]
            if sl not in slot_ps_c:
                slot_ps_c[sl] = (cpsum.tile([128, BPX], F32,
                                            tag="ps_c",
                                            name=f"ps_c{sl}")
                                 if bpad > 1 else None)
                slot_lgas[sl] = []
            blocks = list(range(c0, min(c0 + CPB, bpad)))
            cwid = len(blocks) * BPX
            ps_d = dpsum.tile([128, CW], F32, tag=f"ps_d{(c0 // CPB) % 3}",
                              name=f"ps_d{sl}_{c0}")
            for j, b in enumerate(blocks):
                k = koff[sl] + b
                nc.tensor.matmul(ps_d[:, j * BPX:(j + 1) * BPX],
                                 coef_ap(k),
                                 basis[:], start=True, stop=True)
            # a0 = exp(-D) straight from PSUM; the 0.99 clamp moves to a
            # Pool min (max(D, ln99) pre-exp == min(a0, 0.99) post-exp)
            # and the 1/255 cull is one fused DVE stt on a0 itself.
            a0 = work.tile([128, CW], F32, tag="a0")
            if general:
                # guard: non-PSD D can be hugely negative -> exp inf;
                # clamp D first (slower, correctness-only path)
                Dc = work.tile([128, CW], F32, tag="Dc")
                nc.vector.tensor_scalar(Dc[:, :cwid], ps_d[:, :cwid],
                                        LN99, None, ALU.max)
                nc.scalar.activation(a0[:, :cwid], Dc[:, :cwid],
                                     AF.Exp, scale=-1.0)
            else:
                nc.scalar.activation(a0[:, :cwid], ps_d[:, :cwid],
                                     AF.Exp, scale=-1.0)
            return (sl, blocks, cwid, ps_d, a0)

        def emit_A_back(pend):
            """min/stt/Ln/stair for a front-stage chunk; finishes the slot's
            phase B when it was the slot's last chunk."""
            sl, blocks, cwid, ps_d, a0 = pend
            bpad = bpads[sl]
            ps_c = slot_ps_c[sl]
            lgas = slot_lgas[sl]
            c1 = work.tile([128, CW], F32, tag="c1")
            nc.vector.tensor_scalar(c1[:, :cwid], a0[:, :cwid], 0.99,
                                    None, ALU.min)
            am = work.tile([128, CW], F32, tag="am")
            nc.vector.scalar_tensor_tensor(am[:, :cwid], a0[:, :cwid],
                                           INV255, c1[:, :cwid],
                                           ALU.is_ge, ALU.mult)
            if general:
                am2 = work.tile([128, CW], F32, tag="am2")
                for j, b in enumerate(blocks):
                    k = koff[sl] + b
                    nc.vector.scalar_tensor_tensor(
                        am2[:, j * BPX:(j + 1) * BPX],
                        ps_d[:, j * BPX:(j + 1) * BPX],
                        gs[:, k:k + 1], am[:, j * BPX:(j + 1) * BPX],
                        ALU.is_ge, ALU.mult)
                am = am2
            lga = lgap.tile([128, CW], F32R, tag="lga")
            nc.scalar.activation(lga[:, :cwid], am[:, :cwid], AF.Ln,
                                 scale=-1.0, bias=1.0)
            for j, b in enumerate(blocks):
                lgas.append(lga[:, j * BPX:(j + 1) * BPX])
                if bpad > 1 and b < bpad - 1:
                    nc.tensor.matmul(ps_c[:bpad, :],
                                     st[:, mb * b:mb * b + bpad],
                                     lga[:, j * BPX:(j + 1) * BPX],
                                     start=(b == 0),
                                     stop=(b == bpad - 2))
            if blocks[-1] == bpad - 1:
                # phase B: carries to SBUF (f32, no compensation needed)
                if bpad > 1:
                    cw = carp.tile([128, BPX], F32R, tag="cw")
                    nc.vector.tensor_copy(cw[:bpad, :], ps_c[:bpad, :])
                else:
                    cw = None
                state[sl] = (lgas, cw)

        def emit_C(sl):
            """Phase C + D for slot sl, one chunk per yield."""
            bpad = bpads[sl]
            lgas, cw = state[sl]
            img_ps = ipsum.tile([128, BPX], F32, tag=f"img{sl % 2}",
                                name=f"img{sl}")
            for c0 in range(0, bpad, CPB):
                blocks = list(range(c0, min(c0 + CPB, bpad)))
                cwid = len(blocks) * BPX
                ps_s = spsum.tile([128, CW], F32,
                                  tag=f"scan{(c0 // CPB) % 2}",
                                  name=f"scan{sl}_{c0}")
                for j, b in enumerate(blocks):
                    sseg = ps_s[:, j * BPX:(j + 1) * BPX]
                    nc.tensor.matmul(sseg, u128[:], lgas[b],
                                     start=True, stop=(b == 0))
                    if b > 0:
                        nc.tensor.matmul(sseg,
                                         eb[:bpad, 128 * b:128 * (b + 1)],
                                         cw[:bpad, :],
                                         start=False, stop=True)
                exT = work.tile([128, CW], F32R, tag="exT")
                nc.scalar.activation(exT[:, :cwid], ps_s[:, :cwid], AF.Exp)
                for j, b in enumerate(blocks):
                    k = koff[sl] + b
                    nc.tensor.matmul(img_ps[:3, :], dcw[:, 3 * k:3 * k + 3],
                                     exT[:, j * BPX:(j + 1) * BPX],
                                     start=(b == 0), stop=(b == bpad - 1))
                yield
            ob = outp.tile([128, BPX], F32, tag="ob")
            nc.vector.tensor_copy(ob[:3, :], img_ps[:3, :])
            nc.sync.dma_start(img_d[:, sl * BPX:(sl + 1) * BPX], ob[:3, :])

        # interleaved emission: phase C of slot sl-1 between phase A
        # chunks of slot sl (in-order engine streams get ready work from
        # the other stream while each chunk's serial chain progresses).
        state = {}
        prev_c = None
        pend = None
        for sl in range(NSLOT):
            for c0 in range(0, bpads[sl], CPB):
                p = emit_A_front(sl, c0)
                if pend is not None:
                    emit_A_back(pend)
                pend = p
                if prev_c is not None:
                    next(prev_c, None)
            # finish the slot's last chunk before its phase C can be built
            emit_A_back(pend)
            pend = None
            if prev_c is not None:
                for _ in prev_c:
                    pass
            prev_c = emit_C(sl)
        for _ in prev_c:
            pass

    saved = bacc.get_activation_tables
    bacc.get_activation_tables = _only_full_act_set
    try:
        nc.compile()
    finally:
        bacc.get_activation_tables = saved
    return nc


# ------------------------------------------------------------------ driver
def kernel(context_pose, target_poses, target_intrinsics, means1, means2,
           cov1, cov2, sh1, sh2, op1, op2, background_color,
           image_h, image_w):
    f32 = np.float32
    b, v = np.asarray(target_poses).shape[:2]
    assert b == 1 and v == NCAM and int(image_h) == H and int(image_w) == W

    context_pose = np.asarray(context_pose, f32)
    target_poses = np.asarray(target_poses, f32)
    target_intrinsics = np.asarray(target_intrinsics, f32)
    bg = np.asarray(background_color, f32)

    try:
        inv_base = np.linalg.inv(
            context_pose[0].astype(np.float64)).astype(f32)
    except np.linalg.LinAlgError:
        inv_base = np.linalg.pinv(
            context_pose[0].astype(np.float64)).astype(f32)
    d_sh = np.asarray(sh1).shape[-1]
    means = np.stack([np.asarray(means1, f32), np.asarray(means2, f32)],
                     1).reshape(-1, 3)
    covs = np.stack([np.asarray(cov1, f32), np.asarray(cov2, f32)],
                    1).reshape(-1, 3, 3)
    shs = np.stack([np.asarray(sh1, f32), np.asarray(sh2, f32)],
                   1).reshape(-1, 3, d_sh)
    ops = np.stack([np.asarray(op1, f32), np.asarray(op2, f32)],
                   1).reshape(-1)
    assert means.shape[0] == G

    row_scale = np.array([1.0 / W, 1.0 / H, 1.0], f32)[:, None]

    cams = []
    for cam in range(NCAM):
        extr = inv_base @ target_poses[0, cam]
        Kn = target_intrinsics[0, cam] * row_scale
        K = np.array([[Kn[0, 0] * W, 0, Kn[0, 2] * W],
                      [0, Kn[1, 1] * H, Kn[1, 2] * H],
                      [0, 0, 1]], f32)
        cams.append(_prep_camera(extr, K, bg, means, covs, shs, ops))
    general = not all(c["psd"] for c in cams)

    # cull per rect, then group the 64 rects by survivor count into NSLOT
    # groups of 8 (one per core): slot j runs the j-th largest group, so
    # padding is per-group, not global max.
    pairs = []
    for p in range(NPAIR):
        cam, rest = divmod(p, NBAND * NXS)
        band, xh = divmod(rest, NXS)
        idx, dc, c0 = _cull_rect(cams[cam], band, xh, bg)
        pairs.append((cam, band, xh, idx, dc, c0))
    order = sorted(range(NPAIR), key=lambda p: -len(pairs[p][3]))
    assign = [[order[g * 8 + i] for i in range(8)] for g in range(NSLOT)]
    bpads = tuple(max(1, -(-max(len(pairs[p][3]) for p in grp) // 128))
                  for grp in assign)

    key = (bool(general), bpads)
    if key not in _NC_CACHE:
        _NC_CACHE[key] = _build_nc(general, bpads)
    nc = _NC_CACHE[key]
    mb = max(bpads)
    koff = [sum(bpads[:i]) for i in range(NSLOT)]
    NBLK = sum(bpads)

    # shared constants
    f16 = np.float16
    bx = np.arange(XW, dtype=f32) + 0.5
    px_x = np.tile(bx, BAND_ROWS)
    px_r = np.repeat(np.arange(BAND_ROWS), XW)
    x2 = (px_x * px_x).astype(f32)
    x2h = x2.astype(f16)                       # exact 2-way split (14 bits)
    x2l = (x2 - x2h.astype(f32)).astype(f16)
    basis = np.zeros((NBAS, BPX), f16)
    basis[0] = x2h
    basis[1] = x2l
    basis[2] = x2h
    basis[3] = x2l
    basis[4] = x2h
    for rr in range(BAND_ROWS):
        xr = (px_x * (px_r == rr)).astype(f16)     # x exact in fp16 (7 bits)
        onr = (1.0 * (px_r == rr)).astype(f16)
        for s in range(3):
            basis[5 + 3 * rr + s] = xr
            basis[5 + 3 * BAND_ROWS + 3 * rr + s] = onr
    u128 = np.triu(np.ones((128, 128), f32))                 # k <= j
    st = np.zeros((128, mb * mb), f32)                       # j > b staircase
    for b_ in range(mb):
        st[:, mb * b_ + b_ + 1:mb * (b_ + 1)] = 1.0
    ebm = np.zeros((mb, mb * 128), f32)                      # carry selector
    for b_ in range(mb):
        ebm[b_, b_ * 128:(b_ + 1) * 128] = 1.0

    in_maps = []
    for core in range(8):
        coefs = np.zeros((NBAS, NBLK * 128), np.float16)
        for rr in range(BAND_ROWS):          # pad columns: D = PAD_C, masked
            coefs[5 + 3 * BAND_ROWS + 3 * rr, :] = PAD_C
        dcv = np.zeros((128, NBLK * 3), f32)
        gsv = np.zeros((128, NBLK), f32)
        for slot in range(NSLOT):
            bpad = bpads[slot]
            cam, band, xh, idx, dc, c0 = pairs[assign[slot][core]]
            cp = cams[cam]
            n = len(idx)
            cfull = _coef_block(cp, idx, band, xh)   # [NBAS, n]
            nlg = -cp["logop"][idx]
            for b_ in range(bpad):
                kf = koff[slot] + b_
                lo, hi = b_ * 128, min(n, (b_ + 1) * 128)
                cnt = max(0, hi - lo)
                if cnt > 0:
                    coefs[:, kf * 128:kf * 128 + cnt] = cfull[:, lo:hi]
                    dcv[:cnt, kf * 3:kf * 3 + 3] = dc[lo:hi]
                    gsv[:cnt, kf] = nlg[lo:hi]
        in_maps.append({"coef": coefs, "basis": basis, "dcw": dcv,
                        "u128": u128, "st": st, "eb": ebm, "gs": gsv})

    trace = os.environ.get("SPLAT_TRACE", "0") == "1"
    res = run_bass_kernel_spmd(nc, in_maps, core_ids=list(range(8)),
                               trace=trace,
                               trace_cores=list(range(8)) if trace else None)
    global _LAST_EXEC_NS, _LAST_RESULTS
    _LAST_EXEC_NS = res.exec_time_ns
    _LAST_RESULTS = res

    out = np.zeros((1, NCAM, 3, H, W), f32)
    for core in range(8):
        img = res.results[core]["img"]
        for slot in range(NSLOT):
            cam, band, xh, idx, dc, c0 = pairs[assign[slot][core]]
            piece = img[:, slot * BPX:(slot + 1) * BPX].reshape(
                3, BAND_ROWS, XW)
            out[0, cam, :, band * BAND_ROWS:(band + 1) * BAND_ROWS,
                xh * XW:(xh + 1) * XW] = piece + c0[:, None, None]
    return out
